# revision 31
# baseline (speedup 1.0000x reference)
"""MinibatchDiscrimination kernel for Trainium2 (8 NeuronCores, SPMD).

Math: Ms = (x @ W).reshape(B, 128, 16)
      norm[b,i,j] = sum_d |Ms[b,i,d] - Ms[b,j,d]|
      out[b,i]    = sum_j exp(-norm[b,i,j])

Sharding: data-parallel over batch B across 8 cores (256 samples each).
W replicated (host reorders its columns d-major and pre-casts to bf16).

Pair-route design: the pairwise subtract runs on the TensorEngine.
MsT[k, (d,b)] (kernel-partition layout, via 16 PE transposes of Ms
slices) is the moving operand; a host-built +/-1 "pair difference"
stationary S[k, pair] turns each 128-pair group into one matmul:
  diff[pair, (d,b)] = S.T @ MsT.
Each group's |.|+d-reduction runs either as a DVE tensor_reduce with
apply_absolute_value straight out of PSUM (route A) or as an Act
abs-copy to SBUF followed by a bf16 TT tree at 2x (route B); the A/B
mix is chosen so DVE and Act finish together. exp(-norm) on Act. The
per-kernel sums O[k, b] accumulate on the TensorEngine via 0/1
incidence stationaries (scatter matmuls) into one PSUM bank; a final
PE transpose returns sample-partition layout, +1 adds the excluded
diagonal term exp(0).
"""

import os
import sys

sys.path.insert(0, "/opt/trn_rl_repo")
os.environ.setdefault("MYCRO_LOCAL_CACHE", "1")

import numpy as np
from ml_dtypes import bfloat16

import concourse.bacc as bacc
import concourse.bass as bass
import concourse.tile as tile
from concourse import mybir
from concourse.bass_utils import run_bass_kernel_spmd

B, F, K, D = 2048, 2048, 128, 16
NCORES = 8
BL = B // NCORES          # 256 rows per core
P = 128                   # partitions
NBT = BL // P             # 2 batch tiles per core
FB = F // P               # 16 contraction blocks
ND = K * D                # 2048 matmul output cols
NPAIR = K * (K - 1) // 2  # 8128 unordered pairs
NPG = (NPAIR + P - 1) // P  # 64 pair groups of 128

_BF16 = mybir.dt.bfloat16
_F32 = mybir.dt.float32

# route A (DVE reduce-abs from PSUM) fraction; rest route B (Act
# abs-copy + DVE bf16 tree). Balanced so DVE and Act finish together.
ROUTE_A = [(g % 4) == 0 for g in range(NPG)]


def _pairs():
    out = []
    for i in range(K):
        for j in range(i + 1, K):
            out.append((i, j))
    while len(out) % P:
        out.append(None)
    return out


PAIRS = _pairs()


def _build_nc():
    nc = bacc.Bacc("TRN2", target_bir_lowering=False, debug=False)
    xt = nc.dram_tensor("xt", [F, BL], _BF16, kind="ExternalInput")
    w = nc.dram_tensor("w", [F, ND], _BF16, kind="ExternalInput")
    s_in = nc.dram_tensor("s", [K, NPG * P], _BF16, kind="ExternalInput")
    a_in = nc.dram_tensor("a", [P, NPG * K], _BF16, kind="ExternalInput")
    eye_in = nc.dram_tensor("eye", [P, P], _BF16, kind="ExternalInput")
    out = nc.dram_tensor("out", [BL, K], _F32, kind="ExternalOutput")

    with tile.TileContext(nc) as tc:
        with (
            tc.tile_pool(name="const", bufs=1) as const_pool,
            tc.tile_pool(name="work", bufs=2) as work,
            tc.tile_pool(name="small", bufs=4) as small,
            tc.tile_pool(name="mspsum", bufs=1, space="PSUM") as mspsum,
            tc.tile_pool(name="trpsum", bufs=1, space="PSUM") as trpsum,
            tc.tile_pool(name="prpsum", bufs=5, space="PSUM") as prpsum,
            tc.tile_pool(name="opsum", bufs=1, space="PSUM") as opsum,
        ):
            w_sb = const_pool.tile([P, FB, ND], _BF16)
            xt_sb = const_pool.tile([P, FB, BL], _BF16)
            s_sb = const_pool.tile([K, NPG * P], _BF16)
            a_sb = const_pool.tile([P, NPG * K], _BF16)
            eye = const_pool.tile([P, P], _BF16)
            w_r = w.rearrange("(fb p) n -> p fb n", p=P)
            xt_r = xt.rearrange("(fb p) b -> p fb b", p=P)
            # Spread the 8MB W load over all three DMA-capable queues
            # (serial on one queue it gates the whole ramp ~32us). W
            # d-major: chunk c covers d in [4c, 4c+4) for ALL kernels,
            # so transposes for those d start right after chunk c.
            # s/a head slices (first 16 groups) come early so the first
            # pair-group matmuls and scatters aren't DMA-blocked.
            for fb in range(FB):
                nc.gpsimd.dma_start(out=xt_sb[:, fb, :], in_=xt_r[:, fb, :])
            HC = 16 * P  # s/a head: first 16 groups
            nc.sync.dma_start(out=w_sb[:, :, 0:512], in_=w_r[:, :, 0:512])
            nc.scalar.dma_start(out=s_sb[:, :HC], in_=s_in[:, :HC])
            nc.scalar.dma_start(out=a_sb[:, :HC], in_=a_in[:, :HC])
            nc.scalar.dma_start(
                out=w_sb[:, :, 512:1024], in_=w_r[:, :, 512:1024]
            )
            nc.gpsimd.dma_start(out=eye, in_=eye_in[:, :])
            nc.gpsimd.dma_start(
                out=w_sb[:, :, 1024:1536], in_=w_r[:, :, 1024:1536]
            )
            nc.sync.dma_start(
                out=w_sb[:, :, 1536:2048], in_=w_r[:, :, 1536:2048]
            )
            nc.scalar.dma_start(out=s_sb[:, HC:], in_=s_in[:, HC:])
            nc.sync.dma_start(out=a_sb[:, HC:], in_=a_in[:, HC:])

            ms_tiles = {}

            def _emit_ms_chunk(t, c):
                # ---- Ms = x @ W' (d-major cols), one 512-col chunk ----
                if c == 0:
                    ms_tiles[t] = (
                        work.tile([P, D, K], _BF16, tag="ms", name="ms"),
                        work.tile([K, D, P], _BF16, tag="msT", name="msT"),
                    )
                ms, msT = ms_tiles[t]
                ps = mspsum.tile([P, 512], _F32, tag="msp", name="msp")
                for fb in range(FB):
                    nc.tensor.matmul(
                        ps,
                        xt_sb[:, fb, t * P : (t + 1) * P],
                        w_sb[:, fb, c * 512 : (c + 1) * 512],
                        start=(fb == 0),
                        stop=(fb == FB - 1),
                    )
                nc.scalar.copy(
                    out=ms[:, c * 4 : (c + 1) * 4, :].rearrange(
                        "p d k -> p (d k)"
                    ),
                    in_=ps,
                )
                # transpose the 4 finished d-slices into msT
                tp = trpsum.tile([K, 4, P], _BF16, tag="trp", name="trp")
                for q in range(4):
                    nc.tensor.transpose(tp[:, q, :], ms[:, c * 4 + q, :], eye)
                nc.scalar.copy(
                    out=msT[:, c * 4 : (c + 1) * 4, :].rearrange(
                        "k d b -> k (d b)"
                    ),
                    in_=tp.rearrange("k q b -> k (q b)"),
                )

            for t in range(NBT):
                if t == 0:
                    _emit_ms_chunk(0, 0)
                msT_f = ms_tiles[t][1].rearrange("k d b -> k (d b)")
                o_acc = opsum.tile([K, P], _F32, tag="oacc")
                # ---- pair groups, software-pipelined one group deep:
                # iteration g emits pair-matmuls+abs/tree for group g and
                # exp+scatter for group g-1, so PE never stalls on the
                # group-g round trip through Act/DVE.
                nsrcs = {}

                def _normq(g):
                    bi = g // 4
                    if bi not in nsrcs:
                        nsrcs[bi] = small.tile([P, 4, P], _BF16, tag="normq",
                                               bufs=3, name="normq")
                    return nsrcs[bi]

                fstate = {}

                def _front_chunk(g, c):
                    # one d-quarter: pair matmul into a PSUM 512-chunk,
                    # then drain it (DVE reduce-abs or Act abs-copy)
                    sg = s_sb[:, g * P : (g + 1) * P]
                    pg = prpsum.tile([P, 512], _F32, tag="pg", name="pg")
                    nc.tensor.matmul(
                        pg, sg, msT_f[:, c * 512 : (c + 1) * 512],
                        start=True, stop=True,
                    )
                    if ROUTE_A[g]:
                        nhs = fstate.setdefault(g, [])
                        nh = small.tile([P, P], _F32, tag=f"nh{c}",
                                        name=f"nh{c}")
                        nc.vector.tensor_reduce(
                            out=nh.unsqueeze(2),
                            in_=pg.rearrange(
                                "p (d b) -> p d b", d=4
                            ).transpose([0, 2, 1]),
                            axis=mybir.AxisListType.X,
                            op=mybir.AluOpType.add,
                            apply_absolute_value=True,
                        )
                        nhs.append(nh)
                    else:
                        if g not in fstate:
                            fstate[g] = work.tile([P, D, P], _BF16,
                                                  tag="dvs", bufs=3,
                                                  name="dvs")
                        dvs = fstate[g]
                        nc.scalar.activation(
                            out=dvs[:, c * 4 : (c + 1) * 4, :].rearrange(
                                "p d b -> p (d b)"
                            ),
                            in_=pg,
                            func=mybir.ActivationFunctionType.Abs,
                        )

                def _front_tail(g):
                    if ROUTE_A[g]:
                        nhs = fstate.pop(g)
                        nh01 = small.tile([P, P], _F32, tag="nh01",
                                          name="nh01")
                        nc.vector.tensor_add(nh01, nhs[0], nhs[1])
                        nc.vector.tensor_add(nh01, nh01, nhs[2])
                        nc.vector.tensor_add(
                            _normq(g)[:, g % 4, :], nh01, nhs[3]
                        )
                    else:
                        dvs = fstate.pop(g)
                        l1 = small.tile([P, 8, P], _BF16, tag="l1", name="l1")
                        nc.vector.tensor_add(l1, dvs[:, 0:8, :], dvs[:, 8:16, :])
                        l2 = small.tile([P, 4, P], _BF16, tag="l2", name="l2")
                        nc.vector.tensor_add(l2, l1[:, 0:4, :], l1[:, 4:8, :])
                        l3 = small.tile([P, 2, P], _BF16, tag="l3", name="l3")
                        nc.vector.tensor_add(l3, l2[:, 0:2, :], l2[:, 2:4, :])
                        nc.vector.tensor_add(
                            _normq(g)[:, g % 4, :], l3[:, 0, :], l3[:, 1, :]
                        )

                def _emit_front(g):
                    for c in range(4):
                        _front_chunk(g, c)
                    _front_tail(g)

                eqs = {}

                def _emit_exp(bi):
                    eq = small.tile([P, 4, P], _BF16, tag="eq", bufs=4,
                                    name="eq")
                    nc.scalar.activation(
                        out=eq.rearrange("p q b -> p (q b)"),
                        in_=nsrcs.pop(bi).rearrange("p q b -> p (q b)"),
                        func=mybir.ActivationFunctionType.Exp,
                        scale=-1.0,
                    )
                    eqs[bi] = eq

                def _emit_scatter(bi):
                    # scatter: O[k,b] += sum_pair A[pair,k] * E[pair,b]
                    eq = eqs.pop(bi)
                    for i in range(4):
                        g = bi * 4 + i
                        nc.tensor.matmul(
                            o_acc,
                            a_sb[:, g * K : (g + 1) * K],
                            eq[:, i, :],
                            start=(g == 0),
                            stop=(g == NPG - 1),
                            skip_group_check=True,
                        )

                NB = NPG // 4
                if t == 0:
                    # chunk-major ramp: interleave tile-0 Ms chunks with
                    # batch-0 group chunks so DVE/Act start ~20us earlier
                    for c in range(4):
                        if c > 0:
                            _emit_ms_chunk(0, c)
                        for i in range(4):
                            _front_chunk(i, c)
                    for i in range(4):
                        _front_tail(i)
                for bi in range(NB + 1):
                    if bi < NB and not (t == 0 and bi == 0):
                        for i in range(4):
                            _emit_front(bi * 4 + i)
                    if bi >= 1:
                        _emit_exp(bi - 1)
                        _emit_scatter(bi - 1)
                    # produce the next tile's Ms/MsT during this tile's
                    # late batches so the transition doesn't starve
                    # Act/DVE
                    if t + 1 < NBT and bi in (8, 10, 12, 14):
                        _emit_ms_chunk(t + 1, (bi - 8) // 2)

                # ---- back to sample layout: out[b, k] = o_acc.T + 1 ----
                ob = work.tile([K, P], _BF16, tag="ob")
                nc.scalar.copy(out=ob, in_=o_acc)
                otf = trpsum.tile([K, 4, P], _BF16, tag="trp", name="ot")
                ot = otf[:, 0, :]
                nc.tensor.transpose(ot, ob, eye)
                o2 = work.tile([P, K], _F32, tag="o2")
                nc.vector.tensor_scalar_add(o2, ot, 1.0)
                nc.sync.dma_start(out=out[t * P : (t + 1) * P, :], in_=o2)
    nc.compile()
    return nc


_cached = {}


def _get_nc():
    if "nc" not in _cached:
        _cached["nc"] = _build_nc()
    return _cached["nc"]


def _host_consts():
    s = np.zeros((K, NPG * P), dtype=bfloat16)
    a = np.zeros((P, NPG * K), dtype=bfloat16)
    for p, pr in enumerate(PAIRS):
        if pr is None:
            continue
        i, j = pr
        g, r = divmod(p, P)
        s[i, g * P + r] = 1.0
        s[j, g * P + r] = -1.0
        a[r, g * K + i] = 1.0
        a[r, g * K + j] = 1.0
    eye = np.eye(P, dtype=bfloat16)
    return s, a, eye


def kernel(x: np.ndarray, W: np.ndarray) -> np.ndarray:
    nc = _get_nc()
    xt = np.ascontiguousarray(x.T.astype(bfloat16))  # [F, B]
    # d-major column order: W'[:, (d, k)] = W[:, (k, d)]
    wb = np.ascontiguousarray(
        W.astype(bfloat16).reshape(F, K, D).transpose(0, 2, 1).reshape(F, ND)
    )
    s, a, eye = _host_consts()
    in_maps = [
        {
            "xt": np.ascontiguousarray(xt[:, c * BL : (c + 1) * BL]),
            "w": wb,
            "s": s,
            "a": a,
            "eye": eye,
        }
        for c in range(NCORES)
    ]
    res = run_bass_kernel_spmd(nc, in_maps, core_ids=list(range(NCORES)))
    return np.concatenate(
        [res.results[c]["out"] for c in range(NCORES)], axis=0
    ).astype(np.float32)


# revision 32
# speedup vs baseline: 1.1001x; 1.1001x over previous
"""MinibatchDiscrimination kernel for Trainium2 (8 NeuronCores, SPMD).

Math: Ms = (x @ W).reshape(B, 128, 16)
      norm[b,i,j] = sum_d |Ms[b,i,d] - Ms[b,j,d]|
      out[b,i]    = sum_j exp(-norm[b,i,j])

Sharding: data-parallel over batch B across 8 cores (256 samples each).
W replicated (host reorders its columns d-major and pre-casts to bf16).

Pair-route design: the pairwise subtract runs on the TensorEngine.
MsT[k, (d,b)] (kernel-partition layout, via 16 PE transposes of Ms
slices) is the moving operand; a host-built +/-1 "pair difference"
stationary S[k, pair] turns each 128-pair group into one matmul:
  diff[pair, (d,b)] = S.T @ MsT.
Each group's |.|+d-reduction runs either as a DVE tensor_reduce with
apply_absolute_value straight out of PSUM (route A) or as an Act
abs-copy to SBUF followed by a bf16 TT tree at 2x (route B); the A/B
mix is chosen so DVE and Act finish together. exp(-norm) on Act. The
per-kernel sums O[k, b] accumulate on the TensorEngine via 0/1
incidence stationaries (scatter matmuls) into one PSUM bank; a final
PE transpose returns sample-partition layout, +1 adds the excluded
diagonal term exp(0).
"""

import os
import sys

sys.path.insert(0, "/opt/trn_rl_repo")
os.environ.setdefault("MYCRO_LOCAL_CACHE", "1")

import numpy as np
from ml_dtypes import bfloat16

import concourse.bacc as bacc
import concourse.bass as bass
import concourse.tile as tile
from concourse import mybir
from concourse.bass_utils import run_bass_kernel_spmd

B, F, K, D = 2048, 2048, 128, 16
NCORES = 8
BL = B // NCORES          # 256 rows per core
P = 128                   # partitions
NBT = BL // P             # 2 batch tiles per core
FB = F // P               # 16 contraction blocks
ND = K * D                # 2048 matmul output cols
NPAIR = K * (K - 1) // 2  # 8128 unordered pairs
NPG = (NPAIR + P - 1) // P  # 64 pair groups of 128

_BF16 = mybir.dt.bfloat16
_F32 = mybir.dt.float32

# route A (DVE reduce-abs from PSUM) fraction; rest route B (Act
# abs-copy + DVE bf16 tree). Balanced so DVE and Act finish together.
ROUTE_A = [(g % 4) == 0 for g in range(NPG)]


def _pairs():
    out = []
    for i in range(K):
        for j in range(i + 1, K):
            out.append((i, j))
    while len(out) % P:
        out.append(None)
    return out


PAIRS = _pairs()


def _build_nc():
    nc = bacc.Bacc("TRN2", target_bir_lowering=False, debug=False)
    xt = nc.dram_tensor("xt", [F, BL], _BF16, kind="ExternalInput")
    w = nc.dram_tensor("w", [F, ND], _BF16, kind="ExternalInput")
    s_in = nc.dram_tensor("s", [K, NPG * P], _BF16, kind="ExternalInput")
    a_in = nc.dram_tensor("a", [P, NPG * K], _BF16, kind="ExternalInput")
    eye_in = nc.dram_tensor("eye", [P, P], _BF16, kind="ExternalInput")
    out = nc.dram_tensor("out", [BL, K], _F32, kind="ExternalOutput")

    with tile.TileContext(nc) as tc:
        with (
            tc.tile_pool(name="const", bufs=1) as const_pool,
            tc.tile_pool(name="work", bufs=2) as work,
            tc.tile_pool(name="small", bufs=4) as small,
            tc.tile_pool(name="mspsum", bufs=1, space="PSUM") as mspsum,
            tc.tile_pool(name="trpsum", bufs=1, space="PSUM") as trpsum,
            tc.tile_pool(name="prpsum", bufs=5, space="PSUM") as prpsum,
            tc.tile_pool(name="opsum", bufs=1, space="PSUM") as opsum,
        ):
            w_sb = const_pool.tile([P, FB, ND], _BF16)
            xt_sb = const_pool.tile([P, FB, BL], _BF16)
            s_sb = const_pool.tile([K, NPG * P], _BF16)
            a_sb = const_pool.tile([P, NPG * K], _BF16)
            eye = const_pool.tile([P, P], _BF16)
            w_r = w.rearrange("(fb p) n -> p fb n", p=P)
            xt_r = xt.rearrange("(fb p) b -> p fb b", p=P)
            # Spread the 8MB W load over all three DMA-capable queues
            # (serial on one queue it gates the whole ramp ~32us). W
            # d-major: chunk c covers d in [4c, 4c+4) for ALL kernels,
            # so transposes for those d start right after chunk c.
            # s/a head slices (first 16 groups) come early so the first
            # pair-group matmuls and scatters aren't DMA-blocked.
            for fb in range(FB):
                nc.gpsimd.dma_start(out=xt_sb[:, fb, :], in_=xt_r[:, fb, :])
            HC = 16 * P  # s/a head: first 16 groups
            nc.sync.dma_start(out=w_sb[:, :, 0:512], in_=w_r[:, :, 0:512])
            nc.scalar.dma_start(
                out=w_sb[:, :, 512:1024], in_=w_r[:, :, 512:1024]
            )
            nc.scalar.dma_start(out=s_sb[:, :HC], in_=s_in[:, :HC])
            nc.scalar.dma_start(out=a_sb[:, :HC], in_=a_in[:, :HC])
            nc.gpsimd.dma_start(out=eye, in_=eye_in[:, :])
            nc.gpsimd.dma_start(
                out=w_sb[:, :, 1024:1536], in_=w_r[:, :, 1024:1536]
            )
            nc.sync.dma_start(
                out=w_sb[:, :, 1536:2048], in_=w_r[:, :, 1536:2048]
            )
            # s/a tails are deferred: issued after tile-0's ramp so they
            # don't sit ahead of Act's ramp-critical PSUM copies

            ms_tiles = {}

            def _emit_ms_chunk(t, c):
                # ---- Ms = x @ W' (d-major cols), one 512-col chunk ----
                if c == 0:
                    ms_tiles[t] = (
                        work.tile([P, D, K], _BF16, tag="ms", name="ms"),
                        work.tile([K, D, P], _BF16, tag="msT", name="msT"),
                    )
                ms, msT = ms_tiles[t]
                ps = mspsum.tile([P, 512], _F32, tag="msp", name="msp")
                for fb in range(FB):
                    nc.tensor.matmul(
                        ps,
                        xt_sb[:, fb, t * P : (t + 1) * P],
                        w_sb[:, fb, c * 512 : (c + 1) * 512],
                        start=(fb == 0),
                        stop=(fb == FB - 1),
                    )
                nc.scalar.copy(
                    out=ms[:, c * 4 : (c + 1) * 4, :].rearrange(
                        "p d k -> p (d k)"
                    ),
                    in_=ps,
                )
                # transpose the 4 finished d-slices into msT
                tp = trpsum.tile([K, 4, P], _BF16, tag="trp", name="trp")
                for q in range(4):
                    nc.tensor.transpose(tp[:, q, :], ms[:, c * 4 + q, :], eye)
                nc.scalar.copy(
                    out=msT[:, c * 4 : (c + 1) * 4, :].rearrange(
                        "k d b -> k (d b)"
                    ),
                    in_=tp.rearrange("k q b -> k (q b)"),
                )

            for t in range(NBT):
                if t == 0:
                    _emit_ms_chunk(0, 0)
                msT_f = ms_tiles[t][1].rearrange("k d b -> k (d b)")
                o_acc = opsum.tile([K, P], _F32, tag="oacc")
                # ---- pair groups, software-pipelined one group deep:
                # iteration g emits pair-matmuls+abs/tree for group g and
                # exp+scatter for group g-1, so PE never stalls on the
                # group-g round trip through Act/DVE.
                nsrcs = {}

                def _normq(g):
                    bi = g // 4
                    if bi not in nsrcs:
                        nsrcs[bi] = small.tile([P, 4, P], _BF16, tag="normq",
                                               bufs=3, name="normq")
                    return nsrcs[bi]

                fstate = {}

                def _front_chunk(g, c):
                    # one d-quarter: pair matmul into a PSUM 512-chunk,
                    # then drain it (DVE reduce-abs or Act abs-copy)
                    sg = s_sb[:, g * P : (g + 1) * P]
                    pg = prpsum.tile([P, 512], _F32, tag="pg", name="pg")
                    nc.tensor.matmul(
                        pg, sg, msT_f[:, c * 512 : (c + 1) * 512],
                        start=True, stop=True,
                    )
                    if ROUTE_A[g]:
                        nhs = fstate.setdefault(g, [])
                        nh = small.tile([P, P], _F32, tag=f"nh{c}",
                                        name=f"nh{c}")
                        nc.vector.tensor_reduce(
                            out=nh.unsqueeze(2),
                            in_=pg.rearrange(
                                "p (d b) -> p d b", d=4
                            ).transpose([0, 2, 1]),
                            axis=mybir.AxisListType.X,
                            op=mybir.AluOpType.add,
                            apply_absolute_value=True,
                        )
                        nhs.append(nh)
                    else:
                        if g not in fstate:
                            fstate[g] = work.tile([P, D, P], _BF16,
                                                  tag="dvs", bufs=3,
                                                  name="dvs")
                        dvs = fstate[g]
                        nc.scalar.activation(
                            out=dvs[:, c * 4 : (c + 1) * 4, :].rearrange(
                                "p d b -> p (d b)"
                            ),
                            in_=pg,
                            func=mybir.ActivationFunctionType.Abs,
                        )

                def _front_tail(g):
                    if ROUTE_A[g]:
                        nhs = fstate.pop(g)
                        nh01 = small.tile([P, P], _F32, tag="nh01",
                                          name="nh01")
                        nc.vector.tensor_add(nh01, nhs[0], nhs[1])
                        nc.vector.tensor_add(nh01, nh01, nhs[2])
                        nc.vector.tensor_add(
                            _normq(g)[:, g % 4, :], nh01, nhs[3]
                        )
                    else:
                        dvs = fstate.pop(g)
                        l1 = small.tile([P, 8, P], _BF16, tag="l1", name="l1")
                        nc.vector.tensor_add(l1, dvs[:, 0:8, :], dvs[:, 8:16, :])
                        l2 = small.tile([P, 4, P], _BF16, tag="l2", name="l2")
                        nc.vector.tensor_add(l2, l1[:, 0:4, :], l1[:, 4:8, :])
                        l3 = small.tile([P, 2, P], _BF16, tag="l3", name="l3")
                        nc.vector.tensor_add(l3, l2[:, 0:2, :], l2[:, 2:4, :])
                        nc.vector.tensor_add(
                            _normq(g)[:, g % 4, :], l3[:, 0, :], l3[:, 1, :]
                        )

                def _emit_front(g):
                    for c in range(4):
                        _front_chunk(g, c)
                    _front_tail(g)

                eqs = {}

                def _emit_exp(bi):
                    eq = small.tile([P, 4, P], _BF16, tag="eq", bufs=4,
                                    name="eq")
                    nc.scalar.activation(
                        out=eq.rearrange("p q b -> p (q b)"),
                        in_=nsrcs.pop(bi).rearrange("p q b -> p (q b)"),
                        func=mybir.ActivationFunctionType.Exp,
                        scale=-1.0,
                    )
                    eqs[bi] = eq

                def _emit_scatter(bi):
                    # scatter: O[k,b] += sum_pair A[pair,k] * E[pair,b]
                    eq = eqs.pop(bi)
                    for i in range(4):
                        g = bi * 4 + i
                        nc.tensor.matmul(
                            o_acc,
                            a_sb[:, g * K : (g + 1) * K],
                            eq[:, i, :],
                            start=(g == 0),
                            stop=(g == NPG - 1),
                            skip_group_check=True,
                        )

                NB = NPG // 4
                if t == 0:
                    # chunk-major ramp: interleave tile-0 Ms chunks with
                    # batch-0 group chunks so DVE/Act start ~20us earlier
                    for c in range(4):
                        if c > 0:
                            _emit_ms_chunk(0, c)
                        for i in range(4):
                            _front_chunk(i, c)
                    for i in range(4):
                        _front_tail(i)
                    nc.sync.dma_start(out=s_sb[:, HC:], in_=s_in[:, HC:])
                    nc.sync.dma_start(out=a_sb[:, HC:], in_=a_in[:, HC:])
                for bi in range(NB + 1):
                    if bi < NB and not (t == 0 and bi == 0):
                        for i in range(4):
                            _emit_front(bi * 4 + i)
                    if bi >= 1:
                        _emit_exp(bi - 1)
                        _emit_scatter(bi - 1)
                    # produce the next tile's Ms/MsT during this tile's
                    # late batches so the transition doesn't starve
                    # Act/DVE
                    if t + 1 < NBT and bi in (8, 10, 12, 14):
                        _emit_ms_chunk(t + 1, (bi - 8) // 2)

                # ---- back to sample layout: out[b, k] = o_acc.T + 1 ----
                ob = work.tile([K, P], _BF16, tag="ob")
                nc.scalar.copy(out=ob, in_=o_acc)
                otf = trpsum.tile([K, 4, P], _BF16, tag="trp", name="ot")
                ot = otf[:, 0, :]
                nc.tensor.transpose(ot, ob, eye)
                o2 = work.tile([P, K], _F32, tag="o2")
                nc.vector.tensor_scalar_add(o2, ot, 1.0)
                nc.sync.dma_start(out=out[t * P : (t + 1) * P, :], in_=o2)
    nc.compile()
    return nc


_cached = {}


def _get_nc():
    if "nc" not in _cached:
        _cached["nc"] = _build_nc()
    return _cached["nc"]


def _host_consts():
    s = np.zeros((K, NPG * P), dtype=bfloat16)
    a = np.zeros((P, NPG * K), dtype=bfloat16)
    for p, pr in enumerate(PAIRS):
        if pr is None:
            continue
        i, j = pr
        g, r = divmod(p, P)
        s[i, g * P + r] = 1.0
        s[j, g * P + r] = -1.0
        a[r, g * K + i] = 1.0
        a[r, g * K + j] = 1.0
    eye = np.eye(P, dtype=bfloat16)
    return s, a, eye


def kernel(x: np.ndarray, W: np.ndarray) -> np.ndarray:
    nc = _get_nc()
    xt = np.ascontiguousarray(x.T.astype(bfloat16))  # [F, B]
    # d-major column order: W'[:, (d, k)] = W[:, (k, d)]
    wb = np.ascontiguousarray(
        W.astype(bfloat16).reshape(F, K, D).transpose(0, 2, 1).reshape(F, ND)
    )
    s, a, eye = _host_consts()
    in_maps = [
        {
            "xt": np.ascontiguousarray(xt[:, c * BL : (c + 1) * BL]),
            "w": wb,
            "s": s,
            "a": a,
            "eye": eye,
        }
        for c in range(NCORES)
    ]
    res = run_bass_kernel_spmd(nc, in_maps, core_ids=list(range(NCORES)))
    return np.concatenate(
        [res.results[c]["out"] for c in range(NCORES)], axis=0
    ).astype(np.float32)
